# revision 1
# baseline (speedup 1.0000x reference)
"""MultiHeadLatentAttention on 8 Trainium2 NeuronCores (Bass/Tile, SPMD).

Sharding (tensor parallel over heads, per the hint, plus two refinements):
  - 16 heads / 8 cores = 2 heads per core: q_proj + kv_b_proj output dims and
    o_proj input dim sharded by head.
  - kv_a_proj + rms-norm are token-sharded (512 tokens/core) with an
    AllGather of the normalized latent (small: 1 MB/core) instead of
    replicating the 9.7 GFLOP kv_a matmul on every core.
  - Instead of an AllReduce of full [B,S,H] partial o_proj outputs (33 MB,
    ~380 us), an AllToAll of the attention outputs (4 MB) token-shards the
    o_proj: each core computes the full o_proj for 512 tokens and outputs
    exactly its token slice. Host-side unshard is a pure concat/transpose.

All matmuls run as fp32r (TF32: 10-bit mantissa inputs, fp32 accumulate) at
1 cycle/row on the PE. Inputs are pre-rounded to the TF32 grid on the host;
intermediates are rounded by the producing engine writing float32r.

Layouts keep tokens on the moving/free axis everywhere:
  hsT [hid, tok], qT/kT/vT [d, tok] per (head, batch), scoresT [ktok, qtok]
  (softmax along partitions via ones-matmul), attention out [d, tok],
  o_proj out [out, tok] (host transposes at the end).
"""

import math
from contextlib import ExitStack

import numpy as np

B, S = 2, 2048
T = B * S                     # 4096 flattened tokens
HID = 2048
H, D = 16, 128
RANK, ROPE = 512, 64
MAX_POS, ORIG_POS = 131072, 8192
BASE = 500000.0
BETA_FAST, BETA_SLOW = 32.0, 1.0
EPS = 1e-6
NCORES = 8
HPC = H // NCORES             # 2 heads per core
TPC = T // NCORES             # 512 tokens per core (kv_a shard)
SPC = S // NCORES             # 256 tokens per (core, batch) after AllToAll

_CACHE: dict = {}


def tf32_round(x: np.ndarray) -> np.ndarray:
    u = np.ascontiguousarray(x, dtype=np.float32).view(np.uint32).copy()
    add = ((u >> 13) & 1) + 0xFFF
    u = (u + add) & np.uint32(0xFFFFE000)
    return u.view(np.float32)


def _yarn_cos_sin():
    """cos/sin tables matching reference.py's yarn_cos_sin (mscale folded)."""
    scaling = MAX_POS / ORIG_POS
    pos_freqs = BASE ** (np.arange(0, ROPE, 2, dtype=np.float64) / ROPE)
    extrap = 1.0 / pos_freqs
    interp = 1.0 / (scaling * pos_freqs)
    low = max(math.floor(ROPE * math.log(ORIG_POS / (BETA_FAST * 2 * math.pi))
                         / (2 * math.log(BASE))), 0)
    high = min(math.ceil(ROPE * math.log(ORIG_POS / (BETA_SLOW * 2 * math.pi))
                         / (2 * math.log(BASE))), ROPE - 1)
    i = np.arange(ROPE // 2, dtype=np.float64)
    smooth = np.clip((i - low) / max(high - low, 1), 0.0, 1.0)
    inv_freq = ((1.0 - smooth) * interp + smooth * extrap).astype(np.float32)
    pos = np.arange(S, dtype=np.float32)
    freqs = pos[:, None] * inv_freq[None, :]              # [S, 32]
    emb = np.concatenate([freqs, freqs], axis=-1)         # [S, 64]
    mscale = 0.1 * math.log(scaling) + 1.0
    cos = (np.cos(emb) * mscale).astype(np.float32)
    sin = (np.sin(emb) * mscale).astype(np.float32)
    return cos.T.copy(), sin.T.copy()                     # [64, S] each


def build_nc(passes=1, sim_mode=False):
    """Build + compile the (single, SPMD) Bass program for all 8 cores."""
    import concourse.tile as tile
    import concourse.mybir as mybir
    from concourse import bacc

    F32 = mybir.dt.float32
    F32R = mybir.dt.float32r
    AF = mybir.ActivationFunctionType
    RG = [list(range(NCORES))]

    nc = bacc.Bacc("TRN2", target_bir_lowering=False, debug=False,
                   num_devices=1 if sim_mode else NCORES)

    # ---- kernel I/O ----
    hsT_in = nc.dram_tensor("hsT", [HID, T], F32R, kind="ExternalInput").ap()
    hsmy_in = nc.dram_tensor("hsmy", [HID, TPC], F32R, kind="ExternalInput").ap()
    qwT_in = nc.dram_tensor("qwT", [HID, HPC * D], F32R, kind="ExternalInput").ap()
    kvaT_in = nc.dram_tensor("kvaT", [HID, RANK], F32R, kind="ExternalInput").ap()
    kvbT_in = nc.dram_tensor("kvbT", [RANK, HPC * 2 * D], F32R, kind="ExternalInput").ap()
    owt_in = nc.dram_tensor("owt", [16, 128, HID], F32R, kind="ExternalInput").ap()
    cos_in = nc.dram_tensor("cos", [ROPE, S], F32, kind="ExternalInput").ap()
    sinsh_in = nc.dram_tensor("sinsh", [ROPE, S], F32, kind="ExternalInput").ap()
    ident_in = nc.dram_tensor("ident", [128, 128], F32R, kind="ExternalInput").ap()
    ones_in = nc.dram_tensor("ones", [128, 128], F32R, kind="ExternalInput").ap()
    outTs = [nc.dram_tensor(f"outT{p}" if p else "outT", [HID, 2 * SPC], F32,
                            kind="ExternalOutput").ap() for p in range(passes)]

    NH = HID // 128   # 16 hid chunks
    NR = RANK // 128  # 4 rank chunks

    with tile.TileContext(nc) as tc, ExitStack() as ctx0:
        const = ctx0.enter_context(tc.tile_pool(name="const", bufs=1))
        dram = ctx0.enter_context(tc.tile_pool(name="dram", bufs=1, space="DRAM"))

        ident = const.tile([128, 128], F32R)
        ones = const.tile([128, 128], F32R)
        cosb = const.tile([ROPE, S], F32)
        sinsh = const.tile([ROPE, S], F32)
        eps_t = const.tile([1, 1], F32)
        nc.sync.dma_start(ident[:], ident_in[:])
        nc.sync.dma_start(ones[:], ones_in[:])
        nc.sync.dma_start(cosb[:], cos_in[:])
        nc.sync.dma_start(sinsh[:], sinsh_in[:])
        nc.vector.memset(eps_t[:], EPS)

        for p_ in range(passes):
            # collective bounce buffers
            ag_in = [dram.tile([RANK // 2, TPC], F32R, name=f"agin{p_}{h}")
                     for h in range(2)]
            ag_out = [dram.tile([NCORES, RANK // 2, TPC], F32R,
                                addr_space="Local" if sim_mode else "Shared",
                                name=f"agout{p_}{h}") for h in range(2)]
            a2a_in = [dram.tile([NCORES, HPC * D, SPC], F32R, name=f"a2ain{p_}{b}")
                      for b in range(B)]
            a2a_out = [dram.tile([NCORES, HPC * D, SPC], F32R, name=f"a2aout{p_}{b}")
                       for b in range(B)]

            ctx_pass = ExitStack()
            afp = ctx_pass.enter_context(tc.tile_pool(name=f"afp_{p_}", bufs=1))
            af = afp.tile([128, NH * 2 * SPC], F32R, name=f"af{p_}")
            with ExitStack() as ctx_big:
                big = ctx_big.enter_context(tc.tile_pool(name=f"big_{p_}", bufs=1))
                rope_pool = ctx_big.enter_context(
                    tc.tile_pool(name=f"rope_{p_}", bufs=1))

                def rope_block(X):
                    tmp = rope_pool.tile([ROPE, S], F32, tag="rtmp", bufs=1,
                                         name="rtmp")
                    m2 = rope_pool.tile([ROPE, S], F32, tag="rm2", bufs=1,
                                        name="rm2")
                    nc.vector.tensor_mul(tmp[:], X[0:ROPE], cosb[:])
                    nc.vector.tensor_mul(m2[0:32], X[32:64], sinsh[32:64])
                    nc.vector.tensor_mul(m2[32:64], X[0:32], sinsh[0:32])
                    nc.vector.tensor_add(X[0:ROPE], tmp[:], m2[:])

                # per (head j, batch b) tiles, [128, S] each
                qT = [[big.tile([128, S], F32R, name=f"qT{p_}{j}{b}") for b in range(B)]
                      for j in range(HPC)]
                kT = [[big.tile([128, S], F32R, name=f"kT{p_}{j}{b}") for b in range(B)]
                      for j in range(HPC)]
                vnat = [[big.tile([128, S], F32R, name=f"vn{p_}{j}{b}") for b in range(B)]
                        for j in range(HPC)]

                # ---------- P1: kv_a on my 512-token shard + rms norm + AllGather
                with ExitStack() as c1:
                    p1 = c1.enter_context(tc.tile_pool(name=f"p1_{p_}", bufs=1))
                    p1ps = c1.enter_context(tc.tile_pool(name=f"p1ps_{p_}", bufs=1, space="PSUM"))
                    ps_lat = [p1ps.tile([128, TPC], F32, name=f"pslat{p_}{m}", tag=f"lat{m}")
                              for m in range(NR)]
                    for k in range(NH):
                        kva_t = p1.tile([128, RANK], F32R, tag="kvat", bufs=3)
                        nc.sync.dma_start(kva_t[:], kvaT_in[k * 128:(k + 1) * 128, :])
                        ht = p1.tile([128, TPC], F32R, tag="hsmy", bufs=6)
                        nc.sync.dma_start(ht[:], hsmy_in[k * 128:(k + 1) * 128, :])
                        for m in range(NR):
                            nc.tensor.matmul(
                                ps_lat[m][:],
                                kva_t[:, m * 128:(m + 1) * 128],
                                ht[:], start=(k == 0), stop=(k == NH - 1))
                    # rms norm over rank (partition axis, 4 chunks)
                    lat_sb = p1.tile([128, NR * TPC], F32)
                    ps_var = p1ps.tile([1, TPC], F32, tag="var")
                    for m in range(NR):
                        nc.any.tensor_copy(lat_sb[:, m * TPC:(m + 1) * TPC], ps_lat[m][:])
                    sq = [p1.tile([128, TPC], F32, name=f"sq{p_}{m}", tag="sq", bufs=2)
                          for m in range(NR)]
                    for m in range(NR):
                        nc.vector.tensor_mul(sq[m][:], lat_sb[:, m * TPC:(m + 1) * TPC],
                                             lat_sb[:, m * TPC:(m + 1) * TPC])
                        nc.tensor.matmul(ps_var[:], ones[:, 0:1].bitcast(F32), sq[m][:],
                                         start=(m == 0), stop=(m == NR - 1))
                    std = p1.tile([1, TPC], F32, tag="std")
                    nc.scalar.activation(std[:], ps_var[:], AF.Sqrt,
                                         bias=eps_t[:], scale=1.0 / RANK)
                    istd = p1.tile([1, TPC], F32, tag="istd")
                    nc.vector.reciprocal(istd[:], std[:])
                    ps_bc = p1ps.tile([128, TPC], F32, tag="bc")
                    nc.tensor.matmul(ps_bc[:], ones[0:1, :].bitcast(F32), istd[:],
                                     start=True, stop=True)
                    latn = p1.tile([128, NR * TPC], F32R)
                    for h in range(2):
                        for m2 in range(2):
                            m = 2 * h + m2
                            nc.vector.tensor_mul(latn[:, m * TPC:(m + 1) * TPC],
                                                 lat_sb[:, m * TPC:(m + 1) * TPC],
                                                 ps_bc[:])
                            nc.sync.dma_start(ag_in[h][m2 * 128:(m2 + 1) * 128, :],
                                              latn[:, m * TPC:(m + 1) * TPC])
                        if sim_mode:
                            for s8 in range(NCORES):
                                nc.sync.dma_start(ag_out[h][s8], ag_in[h][:])
                        else:
                            nc.gpsimd.collective_compute(
                                "AllGather", mybir.AluOpType.bypass,
                                replica_groups=RG,
                                ins=[ag_in[h].opt()], outs=[ag_out[h].opt()])

                # ---------- P2: q_proj for my 2 heads over all 4096 tokens
                with ExitStack() as c2:
                    p2 = c2.enter_context(tc.tile_pool(name=f"p2_{p_}", bufs=1))
                    p2ps = c2.enter_context(tc.tile_pool(name=f"p2ps_{p_}", bufs=1, space="PSUM"))
                    for g in range(4):            # 1024-token groups
                        b, half = g // 2, g % 2
                        psq = [[p2ps.tile([128, 512], F32, name=f"psq{p_}{g}{m}{t2}",
                                          tag="psq", bufs=8)
                                for t2 in range(2)] for m in range(HPC)]
                        for k in range(NH):
                            qw_t = p2.tile([128, HPC * D], F32R, tag="qwt", bufs=4)
                            nc.sync.dma_start(qw_t[:],
                                              qwT_in[k * 128:(k + 1) * 128, :])
                            ht = p2.tile([128, 1024], F32R, tag="hsq", bufs=6)
                            nc.sync.dma_start(
                                ht[:], hsT_in[k * 128:(k + 1) * 128,
                                              g * 1024:(g + 1) * 1024])
                            for m in range(HPC):
                                for t2 in range(2):
                                    nc.tensor.matmul(
                                        psq[m][t2][:],
                                        qw_t[:, m * 128:(m + 1) * 128],
                                        ht[:, t2 * 512:(t2 + 1) * 512],
                                        start=(k == 0), stop=(k == NH - 1))
                        for m in range(HPC):
                            for t2 in range(2):
                                col = half * 1024 + t2 * 512
                                nc.any.tensor_copy(qT[m][b][:, col:col + 512],
                                                   psq[m][t2][:])
                        if half == 1:
                            for j in range(HPC):
                                rope_block(qT[j][b])

                # ---------- P3: kv_b for my 2 heads over all tokens (+ v transpose)
                with ExitStack() as c3:
                    p3 = c3.enter_context(tc.tile_pool(name=f"p3_{p_}", bufs=1))
                    p3ps = c3.enter_context(tc.tile_pool(name=f"p3ps_{p_}", bufs=1, space="PSUM"))
                    kvbT_sb = p3.tile([128, NR * HPC * 2 * D], F32R)
                    nc.sync.dma_start(
                        kvbT_sb[:].rearrange("p (r m) -> p r m", r=NR),
                        kvbT_in.rearrange("(r p) m -> p r m", p=128))
                    for tc8 in range(NCORES):     # 512-token chunks (AG layout)
                        b, loc = tc8 // 4, (tc8 % 4) * 512
                        lt = [p3.tile([128, 2 * 512], F32R, tag=f"lt{h}", bufs=4,
                                      name=f"lth{h}") for h in range(2)]
                        for h in range(2):
                            nc.sync.dma_start(
                                lt[h][:].rearrange("p (r t) -> p r t", r=2),
                                ag_out[h][tc8].rearrange("(r p) t -> p r t", p=128))
                        for m in range(2 * HPC):  # k0,v0,k1,v1
                            j, is_v = m // 2, m % 2
                            ps = p3ps.tile([128, 512], F32, tag="kv", bufs=4)
                            for r in range(NR):
                                nc.tensor.matmul(
                                    ps[:],
                                    kvbT_sb[:, r * HPC * 2 * D + m * 128:
                                            r * HPC * 2 * D + (m + 1) * 128],
                                    lt[r // 2][:, (r % 2) * 512:(r % 2 + 1) * 512],
                                    start=(r == 0), stop=(r == NR - 1))
                            if not is_v:
                                nc.any.tensor_copy(kT[j][b][:, loc:loc + 512], ps[:])
                            else:
                                vt = p3.tile([128, 512], F32R, tag="vt", bufs=2)
                                nc.any.tensor_copy(vt[:], ps[:])
                                for q4 in range(4):
                                    pst = p3ps.tile([128, 128], F32R, tag="pst", bufs=2)
                                    nc.tensor.transpose(
                                        pst[:], vt[:, q4 * 128:(q4 + 1) * 128], ident[:])
                                    nc.any.tensor_copy(
                                        vnat[j][b][:, loc + q4 * 128: loc + (q4 + 1) * 128],
                                        pst[:])
                        if tc8 % 4 == 3:
                            for j in range(HPC):
                                rope_block(kT[j][b])

                # ---------- P5: attention per (batch, head), scoresT layout
                with ExitStack() as c5:
                    p5 = c5.enter_context(tc.tile_pool(name=f"p5_{p_}", bufs=1))
                    p5ps = c5.enter_context(tc.tile_pool(name=f"p5ps_{p_}", bufs=1, space="PSUM"))
                    NKT = S // 128   # 16 k-chunks per batch
                    for b in range(B):
                        for j in range(HPC):
                            qt, kt, vn = qT[j][b], kT[j][b], vnat[j][b]
                            for qc in range(4):
                                qs = qt[:, qc * 512:(qc + 1) * 512]
                                ps_av = p5ps.tile([128, 512], F32, tag="av", bufs=2)
                                ps_den = p5ps.tile([128, 512], F32, tag="den", bufs=2)
                                for kp in range(NKT // 2):
                                    ps_s = p5ps.tile([128, 1024], F32, tag="s", bufs=2)
                                    e = p5.tile([128, 1024], F32R, tag="e", bufs=6)
                                    for h2 in range(2):
                                        k16 = 2 * kp + h2
                                        nc.tensor.matmul(
                                            ps_s[:, h2 * 512:(h2 + 1) * 512],
                                            kt[:, k16 * 128:(k16 + 1) * 128], qs,
                                            start=True, stop=True)
                                    nc.scalar.activation(e[:], ps_s[:], AF.Exp)
                                    for h2 in range(2):
                                        k16 = 2 * kp + h2
                                        es = e[:, h2 * 512:(h2 + 1) * 512]
                                        nc.tensor.matmul(
                                            ps_av[:], vn[:, k16 * 128:(k16 + 1) * 128], es,
                                            start=(k16 == 0), stop=(k16 == NKT - 1))
                                        nc.tensor.matmul(
                                            ps_den[0:1, :], ones[:, 0:1], es,
                                            start=(k16 == 0), stop=(k16 == NKT - 1))
                                den_sb = p5.tile([1, 512], F32R, tag="densb", bufs=3)
                                nc.vector.tensor_copy(den_sb[:], ps_den[0:1, :])
                                # broadcast back into the same (now free) den bank
                                nc.tensor.matmul(ps_den[:], ones[0:1, :], den_sb[:],
                                                 start=True, stop=True)
                                rec = p5.tile([128, 512], F32, tag="rec", bufs=3)
                                nc.vector.reciprocal(rec[:], ps_den[:])
                                ao_t = p5.tile([128, 512], F32R, tag="aot", bufs=4)
                                nc.vector.tensor_mul(ao_t[:], ps_av[:], rec[:])
                                for h2a in range(2):
                                    s8 = 2 * qc + h2a
                                    nc.sync.dma_start(
                                        a2a_in[b][s8, j * D:(j + 1) * D, :],
                                        ao_t[:, h2a * SPC:(h2a + 1) * SPC])
                        # AllToAll for this batch as soon as both heads are done
                        if sim_mode:
                            nc.sync.dma_start(a2a_out[b][:], a2a_in[b][:])
                        else:
                            nc.gpsimd.collective_compute(
                                "AllToAll", mybir.AluOpType.bypass, replica_groups=RG,
                                ins=[a2a_in[b].opt()], outs=[a2a_out[b].opt()])
                        for k16 in range(NH):
                            i, halfk = k16 // 2, k16 % 2
                            nc.sync.dma_start(
                                af[:, k16 * 2 * SPC + b * SPC:
                                   k16 * 2 * SPC + (b + 1) * SPC],
                                a2a_out[b][i, halfk * 128:(halfk + 1) * 128, :])

            # ---------- P7: o_proj on my 512 tokens (256 per batch)
            with ExitStack() as c7:
                p7 = c7.enter_context(tc.tile_pool(name=f"p7_{p_}", bufs=1))
                p7ps = c7.enter_context(tc.tile_pool(name=f"p7ps_{p_}", bufs=1, space="PSUM"))
                for om in range(NH):
                    wt = p7.tile([128, HID], F32R, tag="ow", bufs=4)
                    nc.sync.dma_start(wt[:], owt_in[om])
                    ps_o = p7ps.tile([128, 2 * SPC], F32, tag="o", bufs=4)
                    for k16 in range(NH):
                        nc.tensor.matmul(
                            ps_o[:], wt[:, k16 * 128:(k16 + 1) * 128],
                            af[:, k16 * 2 * SPC:(k16 + 1) * 2 * SPC],
                            start=(k16 == 0), stop=(k16 == NH - 1))
                    o_sb = p7.tile([128, 2 * SPC], F32, tag="osb", bufs=3)
                    nc.any.tensor_copy(o_sb[:], ps_o[:])
                    nc.sync.dma_start(outTs[p_][om * 128:(om + 1) * 128, :], o_sb[:])
            ctx_pass.close()

    nc.compile()
    return nc


def build_in_maps(hidden_states, q_w, kv_a_w, kv_b_w, o_w, kv_norm_w):
    hs = np.ascontiguousarray(np.asarray(hidden_states, dtype=np.float32))
    q_w = np.asarray(q_w, dtype=np.float32)
    kv_a_w = np.asarray(kv_a_w, dtype=np.float32)
    kv_b_w = np.asarray(kv_b_w, dtype=np.float32)
    o_w = np.asarray(o_w, dtype=np.float32)
    kv_norm_w = np.asarray(kv_norm_w, dtype=np.float32)

    hsT = tf32_round(np.ascontiguousarray(hs.reshape(T, HID).T))      # [HID, T]
    kvaT = tf32_round(np.ascontiguousarray(kv_a_w[ROPE:, :].T))       # [HID, RANK]
    scale = D ** -0.5
    cosT, sinT = _yarn_cos_sin()
    sinsh = np.concatenate([sinT[32:64], -sinT[0:32]], axis=0)
    ident = np.eye(128, dtype=np.float32)
    ones = np.ones((128, 128), dtype=np.float32)
    # owt[om, p, k*128+m] = o_w[om*128+m, k*128+p]
    owt = tf32_round(np.ascontiguousarray(
        o_w.reshape(16, 128, 16, 128).transpose(0, 3, 2, 1).reshape(16, 128, HID)))

    kvb = (kv_b_w * kv_norm_w[None, :]).reshape(H, 2, D, RANK)

    in_maps = []
    for c in range(NCORES):
        qwT = tf32_round(np.ascontiguousarray(
            (q_w[c * HPC * D:(c + 1) * HPC * D] * scale).T))           # [HID, 256]
        # kvbT rows order per core: k0,v0,k1,v1 each 128 wide
        blk = kvb[c * HPC:(c + 1) * HPC]                               # [2,2,128,RANK]
        kvbT = tf32_round(np.ascontiguousarray(
            blk.reshape(HPC * 2 * D, RANK).T))                         # [RANK, 512]
        hsmy = tf32_round(np.ascontiguousarray(
            hsT[:, c * TPC:(c + 1) * TPC]))
        in_maps.append({
            "hsT": hsT, "hsmy": hsmy, "qwT": qwT, "kvaT": kvaT,
            "kvbT": kvbT, "owt": owt, "cos": cosT, "sinsh": sinsh,
            "ident": ident, "ones": ones,
        })
    return in_maps


def assemble_output(results):
    out = np.empty((B, S, HID), dtype=np.float32)
    for c in range(NCORES):
        r = results[c]["outT"]                 # [HID, 2*SPC]
        out[0, c * SPC:(c + 1) * SPC, :] = r[:, 0:SPC].T
        out[1, c * SPC:(c + 1) * SPC, :] = r[:, SPC:2 * SPC].T
    return out


def kernel(hidden_states, q_w, kv_a_w, kv_b_w, o_w, kv_norm_w):
    from concourse import bass_utils

    if "nc" not in _CACHE:
        _CACHE["nc"] = build_nc()
    nc = _CACHE["nc"]
    in_maps = build_in_maps(hidden_states, q_w, kv_a_w, kv_b_w, o_w, kv_norm_w)
    res = bass_utils.run_bass_kernel_spmd(
        nc, in_maps, core_ids=list(range(NCORES)), trace=False)
    return assemble_output(res.results)



# revision 48
# speedup vs baseline: 2.8169x; 2.8169x over previous
"""MultiHeadLatentAttention on 8 Trainium2 NeuronCores (Bass/Tile, SPMD).

Sharding (tensor parallel over heads, plus refinements):
  - 16 heads / 8 cores = 2 heads per core: q_proj + kv_b_proj output dims and
    o_proj input dim sharded by head.
  - kv_a_proj + rms-norm are token-sharded (512 tokens/core) with an
    AllGather of the normalized latent instead of replicating the kv_a
    matmul on every core.
  - An AllToAll of the attention outputs token-shards the o_proj: each core
    computes the full o_proj for 512 tokens and outputs exactly its token
    slice. Host-side unshard is a pure concat/transpose.

All matmul inputs are bfloat16 (fp32 PSUM accumulation): same PE speed as
fp32r (1 cycle/row) but half the DMA/SBUF traffic. Error budget is wide
(gate 2e-2; bf16 lands ~1e-3).

Attention structure (vs a fp32r baseline that spent ~55us of PE on
softmax-denominator ones-matmuls):
  - scores: stationary kT chunk [d,128k], moving q [d,512q] -> sT [k, q].
  - exp on ACT -> e [k, q] bf16.
  - AV is FLIPPED: stationary e-block [128k, 128q], moving V in natural
    [tok, d] layout with a ones-column appended -> out [q, d+1] where
    column d is the softmax denominator, computed for free by the same
    matmul stream.
  - kv_b's V half is also flipped (stationary latent blocks, moving
    kv_b_w v-columns) so V is produced directly as [tok, d] - no separate
    PE transpose / double-copy of V.
  - normalize = DVE tensor_scalar_mul with per-partition 1/den, then one
    PE transpose back to [d, tok] for the AllToAll layout.

Overlap: q_proj group 0 is emitted before kv_a so the PE starts
immediately; the kv_a rms-norm tail hides under q_proj group 1; the
AllGather hides under q_proj groups 1-3; AllToAlls are split per
(batch, head) and o_proj(b=0) is interleaved into attention(b=1).
"""

import math
from contextlib import ExitStack

import numpy as np

B, S = 2, 2048
T = B * S                     # 4096 flattened tokens
HID = 2048
H, D = 16, 128
RANK, ROPE = 512, 64
MAX_POS, ORIG_POS = 131072, 8192
BASE = 500000.0
BETA_FAST, BETA_SLOW = 32.0, 1.0
EPS = 1e-6
NCORES = 8
HPC = H // NCORES             # 2 heads per core
TPC = T // NCORES             # 512 tokens per core (kv_a shard)
SPC = S // NCORES             # 256 tokens per (core, batch) after AllToAll
DP1 = D + 1                   # V width incl. ones column (denominator)

_CACHE: dict = {}


def _yarn_cos_sin():
    """cos/sin tables matching reference.py's yarn_cos_sin (mscale folded)."""
    scaling = MAX_POS / ORIG_POS
    pos_freqs = BASE ** (np.arange(0, ROPE, 2, dtype=np.float64) / ROPE)
    extrap = 1.0 / pos_freqs
    interp = 1.0 / (scaling * pos_freqs)
    low = max(math.floor(ROPE * math.log(ORIG_POS / (BETA_FAST * 2 * math.pi))
                         / (2 * math.log(BASE))), 0)
    high = min(math.ceil(ROPE * math.log(ORIG_POS / (BETA_SLOW * 2 * math.pi))
                         / (2 * math.log(BASE))), ROPE - 1)
    i = np.arange(ROPE // 2, dtype=np.float64)
    smooth = np.clip((i - low) / max(high - low, 1), 0.0, 1.0)
    inv_freq = ((1.0 - smooth) * interp + smooth * extrap).astype(np.float32)
    pos = np.arange(S, dtype=np.float32)
    freqs = pos[:, None] * inv_freq[None, :]              # [S, 32]
    emb = np.concatenate([freqs, freqs], axis=-1)         # [S, 64]
    mscale = 0.1 * math.log(scaling) + 1.0
    cos = (np.cos(emb) * mscale).astype(np.float32)
    sin = (np.sin(emb) * mscale).astype(np.float32)
    return cos.T.copy(), sin.T.copy()                     # [64, S] each


def build_nc(passes=1, sim_mode=False):
    """Build + compile the (single, SPMD) Bass program for all 8 cores."""
    import concourse.tile as tile
    import concourse.mybir as mybir
    from concourse import bacc

    F32 = mybir.dt.float32
    BF16 = mybir.dt.bfloat16
    AF = mybir.ActivationFunctionType
    RG = [list(range(NCORES))]

    nc = bacc.Bacc("TRN2", target_bir_lowering=False, debug=False,
                   num_devices=1 if sim_mode else NCORES)

    # ---- kernel I/O ----
    hsT_in = nc.dram_tensor("hsT", [HID, T], BF16, kind="ExternalInput").ap()
    hsmy_in = nc.dram_tensor("hsmy", [HID, TPC], BF16, kind="ExternalInput").ap()
    qwT_in = nc.dram_tensor("qwT", [HID, HPC * D], BF16, kind="ExternalInput").ap()
    kvaT_in = nc.dram_tensor("kvaT", [HID, RANK], BF16, kind="ExternalInput").ap()
    kvbT_in = nc.dram_tensor("kvbT", [RANK, HPC * 2 * D], BF16, kind="ExternalInput").ap()
    owt_in = nc.dram_tensor("owt", [16, 128, HID], BF16, kind="ExternalInput").ap()
    cos_in = nc.dram_tensor("cos", [ROPE, S], BF16, kind="ExternalInput").ap()
    sinsh_in = nc.dram_tensor("sinsh", [ROPE, S], BF16, kind="ExternalInput").ap()
    ident_in = nc.dram_tensor("ident", [128, 128], BF16, kind="ExternalInput").ap()
    ones_in = nc.dram_tensor("ones", [128, 128], BF16, kind="ExternalInput").ap()
    outTs = [nc.dram_tensor(f"outT{p}" if p else "outT", [HID, 2 * SPC], F32,
                            kind="ExternalOutput").ap() for p in range(passes)]

    NH = HID // 128   # 16 hid chunks
    NR = RANK // 128  # 4 rank chunks
    NKT = S // 128    # 16 k-chunks per batch

    with tile.TileContext(nc) as tc, ExitStack() as ctx0:
        const = ctx0.enter_context(tc.tile_pool(name="const", bufs=1))
        dram = ctx0.enter_context(tc.tile_pool(name="dram", bufs=1, space="DRAM"))

        ident = const.tile([128, 128], BF16)
        ones = const.tile([128, 128], BF16)
        cosb = const.tile([ROPE, S], BF16)
        sinsh = const.tile([ROPE, S], BF16)
        eps_t = const.tile([1, 1], F32)
        nc.vector.memset(eps_t[:], EPS)

        def load_consts():
            nc.sync.dma_start(ident[:], ident_in[:])
            nc.sync.dma_start(ones[:], ones_in[:])
            nc.sync.dma_start(cosb[:], cos_in[:])
            nc.sync.dma_start(sinsh[:], sinsh_in[:])

        for p_ in range(passes):
            # collective bounce buffers
            ag_in = [dram.tile([RANK // 2, TPC], BF16, name=f"agin{p_}{h}")
                     for h in range(2)]
            ag_out = [dram.tile([NCORES, RANK // 2, TPC], BF16,
                                addr_space="Local" if sim_mode else "Shared",
                                name=f"agout{p_}{h}") for h in range(2)]
            a2a_in = [[dram.tile([NCORES, D, SPC], BF16, name=f"a2ain{p_}{b}{j}")
                       for j in range(HPC)] for b in range(B)]
            a2a_out = [[dram.tile([NCORES, D, SPC], BF16,
                                  name=f"a2aout{p_}{b}{j}")
                        for j in range(HPC)] for b in range(B)]

            ctx_pass = ExitStack()
            afp = ctx_pass.enter_context(tc.tile_pool(name=f"afp_{p_}", bufs=1))
            af = afp.tile([128, NH * 2 * SPC], BF16, name=f"af{p_}")
            with ExitStack() as ctx_big:
                big = ctx_big.enter_context(tc.tile_pool(name=f"big_{p_}", bufs=1))
                crp = ctx_big.enter_context(ExitStack())
                rope_pool = crp.enter_context(
                    tc.tile_pool(name=f"rope_{p_}", bufs=1))

                def rope_block(X):
                    tmp = rope_pool.tile([ROPE, S], BF16, tag="rtmp", bufs=1,
                                         name="rtmp")
                    m2 = rope_pool.tile([ROPE, S], BF16, tag="rm2", bufs=1,
                                        name="rm2")
                    nc.vector.tensor_mul(tmp[:], X[0:ROPE], cosb[:])
                    nc.vector.tensor_mul(m2[0:32], X[32:64], sinsh[32:64])
                    nc.vector.tensor_mul(m2[32:64], X[0:32], sinsh[0:32])
                    nc.vector.tensor_add(X[0:ROPE], tmp[:], m2[:])

                # per (head j, batch b) tiles
                qT = [[big.tile([128, S], BF16, name=f"qT{p_}{j}{b}")
                       for b in range(B)] for j in range(HPC)]
                kT = [[big.tile([128, S], BF16, name=f"kT{p_}{j}{b}")
                       for b in range(B)] for j in range(HPC)]
                # V in natural [tok, d] layout, 16 chunks of [128, 129]
                # (col 128 of each chunk = 1.0 -> denominator column)
                vnat = [[big.tile([128, NKT * DP1], BF16, name=f"vn{p_}{j}{b}")
                         for b in range(B)] for j in range(HPC)]
                for j in range(HPC):
                    for b in range(B):
                        nc.vector.memset(
                            vnat[j][b][:].rearrange("p (c w) -> p c w",
                                                    w=DP1)[:, :, D:DP1], 1.0)

                # P1 weights/activations pool (loads emitted after q_proj g0
                # so g0's hsT tiles win the DMA queue)
                c1w = ctx_big.enter_context(ExitStack())
                p1w = c1w.enter_context(tc.tile_pool(name=f"p1w_{p_}", bufs=1))
                kva_sb = p1w.tile([128, NH * RANK], BF16)
                hsm_sb = p1w.tile([128, NH * TPC], BF16)

                # ---------- P2 (part): q_proj — shared pool, emitted in groups
                c2 = ctx_big.enter_context(ExitStack())
                p2 = c2.enter_context(tc.tile_pool(name=f"p2_{p_}", bufs=1))
                p2ps = c2.enter_context(tc.tile_pool(name=f"p2ps_{p_}", bufs=1,
                                                     space="PSUM"))
                qw_sb = p2.tile([128, NH * HPC * D], BF16)
                nc.sync.dma_start(
                    qw_sb[:].rearrange("p (k m) -> p k m", k=NH),
                    qwT_in.rearrange("(k p) m -> p k m", p=128))

                def qproj_group(g):
                    b, half = g // 2, g % 2
                    psq = [[p2ps.tile([128, 512], F32, name=f"psq{p_}{g}{m}{t2}",
                                      tag="psq", bufs=4)
                            for t2 in range(2)] for m in range(HPC)]
                    for k4 in range(NH // 4):
                        ht = p2.tile([128, 4 * 1024], BF16, tag="hsq", bufs=3)
                        if g == 0:
                            # chunk-granular so the first matmul starts ASAP
                            for c4 in range(4):
                                nc.sync.dma_start(
                                    ht[:, c4 * 1024:(c4 + 1) * 1024],
                                    hsT_in[(k4 * 4 + c4) * 128:
                                           (k4 * 4 + c4 + 1) * 128, 0:1024])
                        else:
                            nc.sync.dma_start(
                                ht[:].rearrange("p (c t) -> p c t", c=4),
                                hsT_in[k4 * 512:(k4 + 1) * 512,
                                       g * 1024:(g + 1) * 1024]
                                .rearrange("(c p) t -> p c t", p=128))
                        for c4 in range(4):
                            k = k4 * 4 + c4
                            for m in range(HPC):
                                for t2 in range(2):
                                    nc.tensor.matmul(
                                        psq[m][t2][:],
                                        qw_sb[:, k * HPC * D + m * 128:
                                              k * HPC * D + (m + 1) * 128],
                                        ht[:, c4 * 1024 + t2 * 512:
                                           c4 * 1024 + (t2 + 1) * 512],
                                        start=(k == 0), stop=(k == NH - 1))
                    for m in range(HPC):
                        for t2 in range(2):
                            col = half * 1024 + t2 * 512
                            nc.vector.tensor_copy(qT[m][b][:, col:col + 512],
                                                  psq[m][t2][:])
                    if half == 1:
                        for j in range(HPC):
                            rope_block(qT[j][b])

                qproj_group(0)
                nc.sync.dma_start(
                    kva_sb[:].rearrange("p (k r) -> p k r", k=NH),
                    kvaT_in.rearrange("(k p) r -> p k r", p=128))
                nc.sync.dma_start(
                    hsm_sb[:].rearrange("p (k t) -> p k t", k=NH),
                    hsmy_in.rearrange("(k p) t -> p k t", p=128))
                if p_ == 0:
                    load_consts()
                qproj_group(1)

                # ---------- P1: kv_a on my 512-token shard (2-bank two-pass)
                c1 = ctx_big.enter_context(ExitStack())
                p1 = c1.enter_context(tc.tile_pool(name=f"p1_{p_}", bufs=1))
                p1ps = c1.enter_context(tc.tile_pool(name=f"p1ps_{p_}", bufs=1,
                                                     space="PSUM"))
                lat_sb = p1.tile([128, NR * TPC], F32)
                for half2 in range(2):           # rank halves (2 chunks each)
                    ps_lat = [p1ps.tile([128, TPC], F32, tag="lat", bufs=2,
                                        name=f"pslat{p_}{half2}{m}")
                              for m in range(2)]
                    for k in range(NH):
                        for m in range(2):
                            nc.tensor.matmul(
                                ps_lat[m][:],
                                kva_sb[:, k * RANK + half2 * 256 + m * 128:
                                       k * RANK + half2 * 256 + (m + 1) * 128],
                                hsm_sb[:, k * TPC:(k + 1) * TPC],
                                start=(k == 0), stop=(k == NH - 1))
                    for m in range(2):
                        mg = half2 * 2 + m
                        nc.any.tensor_copy(lat_sb[:, mg * TPC:(mg + 1) * TPC],
                                           ps_lat[m][:])

                sq = [p1.tile([128, TPC], BF16, name=f"sq{p_}{m}", tag="sq", bufs=2)
                      for m in range(NR)]
                for m in range(NR):
                    nc.vector.tensor_mul(sq[m][:], lat_sb[:, m * TPC:(m + 1) * TPC],
                                         lat_sb[:, m * TPC:(m + 1) * TPC])

                qproj_group(2)

                # rms-norm tail + AllGather (hidden under q_proj group 2)
                ps_var = p1ps.tile([1, TPC], F32, tag="var")
                for m in range(NR):
                    nc.tensor.matmul(ps_var[:], ones[:, 0:1], sq[m][:],
                                     start=(m == 0), stop=(m == NR - 1))
                std = p1.tile([1, TPC], F32, tag="std")
                nc.scalar.activation(std[:], ps_var[:], AF.Sqrt,
                                     bias=eps_t[:], scale=1.0 / RANK)
                istd = p1.tile([1, TPC], BF16, tag="istd")
                with nc.allow_low_precision(reason="bf16 istd feeds matmul"):
                    nc.vector.reciprocal(istd[:], std[:])
                ps_bc = p1ps.tile([128, TPC], F32, tag="bc")
                nc.tensor.matmul(ps_bc[:], ones[0:1, :], istd[:],
                                 start=True, stop=True)
                latn = p1.tile([128, NR * TPC], BF16)
                for h in range(2):
                    for m2 in range(2):
                        m = 2 * h + m2
                        nc.vector.tensor_mul(latn[:, m * TPC:(m + 1) * TPC],
                                             lat_sb[:, m * TPC:(m + 1) * TPC],
                                             ps_bc[:])
                        nc.sync.dma_start(ag_in[h][m2 * 128:(m2 + 1) * 128, :],
                                          latn[:, m * TPC:(m + 1) * TPC])
                    if sim_mode:
                        for s8 in range(NCORES):
                            nc.sync.dma_start(ag_out[h][s8], ag_in[h][:])
                    else:
                        nc.gpsimd.collective_compute(
                            "AllGather", mybir.AluOpType.bypass,
                            replica_groups=RG,
                            ins=[ag_in[h].opt()], outs=[ag_out[h].opt()])
                c1.close()

                qproj_group(3)
                c2.close()
                c1w.close()

                # ---------- P3: kv_b for my 2 heads over all tokens
                #   K half direct: stat kv_b k-cols, moving latent -> [d, tok]
                #   V half flipped: stat latent blocks, moving kv_b v-cols
                #   -> V directly in [tok, d] layout (both heads per matmul)
                with ExitStack() as c3:
                    p3 = c3.enter_context(tc.tile_pool(name=f"p3_{p_}", bufs=1))
                    p3ps = c3.enter_context(tc.tile_pool(name=f"p3ps_{p_}", bufs=1, space="PSUM"))
                    kvbT_sb = p3.tile([128, NR * HPC * 2 * D], BF16)
                    nc.sync.dma_start(
                        kvbT_sb[:].rearrange("p (r m) -> p r m", r=NR),
                        kvbT_in.rearrange("(r p) m -> p r m", p=128))
                    for tc8 in range(NCORES):     # 512-token chunks (AG layout)
                        b, loc = tc8 // 4, (tc8 % 4) * 512
                        lt = [p3.tile([128, 2 * 512], BF16, tag=f"lt{h}", bufs=3,
                                      name=f"lth{h}") for h in range(2)]
                        for h in range(2):
                            nc.sync.dma_start(
                                lt[h][:].rearrange("p (r t) -> p r t", r=2),
                                ag_out[h][tc8].rearrange("(r p) t -> p r t", p=128))
                        # K: per head, accumulate over rank chunks
                        for j in range(HPC):
                            ps = p3ps.tile([128, 512], F32, tag="kv", bufs=2)
                            for r in range(NR):
                                nc.tensor.matmul(
                                    ps[:],
                                    kvbT_sb[:, r * 512 + j * 128:
                                            r * 512 + (j + 1) * 128],
                                    lt[r // 2][:, (r % 2) * 512:(r % 2 + 1) * 512],
                                    start=(r == 0), stop=(r == NR - 1))
                            nc.scalar.copy(kT[j][b][:, loc:loc + 512], ps[:])
                        # V flipped: out [tok, 2 heads * d]
                        ps_v = p3ps.tile([128, 4 * 2 * D], F32, tag="vv", bufs=2)
                        for t4 in range(4):
                            for r in range(NR):
                                nc.tensor.matmul(
                                    ps_v[:, t4 * 2 * D:(t4 + 1) * 2 * D],
                                    lt[r // 2][:, (r % 2) * 512 + t4 * 128:
                                               (r % 2) * 512 + (t4 + 1) * 128],
                                    kvbT_sb[:, r * 512 + 2 * D:(r + 1) * 512],
                                    start=(r == 0), stop=(r == NR - 1))
                        for t4 in range(4):
                            cch = (loc // 128) + t4
                            for j in range(HPC):
                                nc.scalar.copy(
                                    vnat[j][b][:, cch * DP1:cch * DP1 + D],
                                    ps_v[:, t4 * 2 * D + j * D:
                                         t4 * 2 * D + (j + 1) * D])
                        if tc8 % 4 == 3:
                            for j in range(HPC):
                                rope_block(kT[j][b])
                crp.close()

                # ---------- P5: attention; o_proj(b=0) interleaved into b=1
                p7 = ctx_big.enter_context(tc.tile_pool(name=f"p7_{p_}", bufs=1))
                owt_sb = p7.tile([128, NH * HID], BF16)
                nc.sync.dma_start(
                    owt_sb[:].rearrange("p (o m) -> p o m", o=NH),
                    owt_in.rearrange("o p m -> p o m"))

                with ExitStack() as c5:
                    p5 = c5.enter_context(tc.tile_pool(name=f"p5_{p_}", bufs=1))
                    p5ps = c5.enter_context(tc.tile_pool(name=f"p5ps_{p_}", bufs=1, space="PSUM"))

                    def oproj_unit(b, om, phase=None, o_ev=None):
                        """o_proj for one 128-row output chunk of batch b.
                        phase None: full contraction. phase 0: even k16
                        (head j=0 features) into o_ev. phase 1: odd k16 +
                        add o_ev, write out."""
                        # shares the p5ps "av" ring (see av_norm)
                        ps_o = p5ps.tile([128, 2 * DP1], F32, tag="av",
                                         bufs=3, name="pso")[:, 0:SPC]
                        ks = (range(NH) if phase is None
                              else range(phase, NH, 2))
                        for i, k16 in enumerate(ks):
                            nc.tensor.matmul(
                                ps_o[:],
                                owt_sb[:, om * HID + k16 * 128:
                                       om * HID + (k16 + 1) * 128],
                                af[:, k16 * 2 * SPC + b * SPC:
                                   k16 * 2 * SPC + (b + 1) * SPC],
                                start=(i == 0), stop=(k16 == max(ks)))
                        if phase == 0:
                            nc.vector.tensor_copy(o_ev[:], ps_o[:])
                            return
                        o_sb = p7.tile([128, SPC], F32, tag="osb", bufs=2)
                        if phase == 1:
                            nc.vector.tensor_add(o_sb[:], ps_o[:], o_ev[:])
                        else:
                            nc.vector.tensor_copy(o_sb[:], ps_o[:])
                        nc.sync.dma_start(
                            outTs[p_][om * 128:(om + 1) * 128,
                                      b * SPC:(b + 1) * SPC], o_sb[:])

                    def scores_pair(b, j, qc, e, kp):
                        """One [128,1024] scores tile + its exp into e."""
                        qs = qT[j][b][:, qc * 512:(qc + 1) * 512]
                        kt = kT[j][b]
                        ps_s = p5ps.tile([128, 1024], F32, tag="s", bufs=2)
                        for h2 in range(2):
                            k16 = 2 * kp + h2
                            nc.tensor.matmul(
                                ps_s[:, h2 * 512:(h2 + 1) * 512],
                                kt[:, k16 * 128:(k16 + 1) * 128], qs,
                                start=True, stop=True)
                        nc.scalar.activation(e[:, kp * 1024:(kp + 1) * 1024],
                                             ps_s[:], AF.Exp)

                    def scores_exp(b, j, qc):
                        """scores + exp for 512 q-tokens; returns the e tile."""
                        e = p5.tile([128, NKT * 512], BF16, tag="e", bufs=3)
                        for kp in range(NKT // 2):
                            scores_pair(b, j, qc, e, kp)
                        return e

                    def av_norm(b, j, qc, e, nxt=None):
                        """AV (flipped, fused denominator) + normalize +
                        transpose + a2a_in staging for 512 q-tokens. The next
                        unit's scores/exp (nxt = (b,j,qc,e)) are interleaved
                        between AV blocks so ACT never starves."""
                        vn = vnat[j][b]
                        ps_av = [p5ps.tile([128, 2 * DP1], F32, tag="av",
                                           bufs=3, name=f"psav{i}")
                                 for i in range(2)]
                        # k-outer so AV consumes e chunks as exp produces them
                        # (overlapping this unit's own exp stream). Only one
                        # pending accumulation group per PSUM bank is legal,
                        # so interleave across banks: q4 0,2 then q4 1,3.
                        for half in range(2):
                            for k16 in range(NKT):
                                for q4 in (half, 2 + half):
                                    nc.tensor.matmul(
                                        ps_av[q4 // 2][:, (q4 % 2) * DP1:
                                                       (q4 % 2 + 1) * DP1],
                                        e[:, k16 * 512 + q4 * 128:
                                          k16 * 512 + (q4 + 1) * 128],
                                        vn[:, k16 * DP1:(k16 + 1) * DP1],
                                        start=(k16 == 0), stop=(k16 == NKT - 1))
                                if (nxt is not None and half == 0
                                        and k16 % 2 == 1):
                                    scores_pair(nxt[0], nxt[1], nxt[2], nxt[3],
                                                k16 // 2)
                        # evacuate av to SBUF (frees the PSUM ring fast),
                        # normalize on the idle Pool engine, transpose on PE
                        pst = p5ps.tile([128, 256], BF16, tag="pst", bufs=1)
                        aot = p5.tile([128, 512], BF16, tag="aot", bufs=2)
                        av_sb = p5.tile([128, 4 * DP1], F32, tag="avsb", bufs=2)
                        for q4 in range(4):
                            nc.vector.tensor_copy(
                                av_sb[:, q4 * DP1:(q4 + 1) * DP1],
                                ps_av[q4 // 2][:, (q4 % 2) * DP1:
                                               (q4 % 2 + 1) * DP1])
                        rec = p5.tile([128, 4], F32, tag="rec", bufs=2)
                        nc.vector.reciprocal(
                            rec[:], av_sb[:].rearrange("p (c w) -> p c w",
                                                       w=DP1)[:, :, D:DP1])
                        for q4 in range(4):
                            ao = p5.tile([128, 128], BF16, tag="ao", bufs=2)
                            with nc.allow_low_precision(reason="attn out bf16"):
                                nc.vector.tensor_scalar_mul(
                                    ao[:], av_sb[:, q4 * DP1:q4 * DP1 + D],
                                    rec[:, q4:q4 + 1])
                            nc.tensor.transpose(
                                pst[:, (q4 % 2) * 128:(q4 % 2 + 1) * 128],
                                ao[:], ident[:])
                            if q4 % 2 == 1:
                                nc.vector.tensor_copy(
                                    aot[:, (q4 // 2) * 256:(q4 // 2 + 1) * 256],
                                    pst[:])
                        nc.sync.dma_start(
                            a2a_in[b][j][2 * qc:2 * qc + 2]
                            .rearrange("s p t -> p s t"),
                            aot[:].rearrange("p (s t) -> p s t", s=2))

                    def a2a_and_af(b, j):
                        if sim_mode:
                            nc.sync.dma_start(a2a_out[b][j][:], a2a_in[b][j][:])
                        else:
                            nc.gpsimd.collective_compute(
                                "AllToAll", mybir.AluOpType.bypass, replica_groups=RG,
                                ins=[a2a_in[b][j].opt()], outs=[a2a_out[b][j].opt()])
                        off = j * 2 * SPC + b * SPC
                        nc.sync.dma_start(
                            af[:].rearrange("p (i r) -> p i r",
                                            i=NCORES)[:, :, off:off + SPC],
                            a2a_out[b][j].rearrange("i p t -> p i t"))

                    # Software-pipelined attention: scores/exp of unit n+1 are
                    # emitted before AV of unit n, so ACT (exp) overlaps the
                    # PE's AV + o_proj work. o_proj(b=0) is interleaved into
                    # batch 1 (skip the first 2 units so o_proj doesn't stall
                    # the PE on the A2A).
                    units = [(b, j, qc) for b in range(B) for j in range(HPC)
                             for qc in range(4)]
                    om_per_unit = {8 + u: n
                                   for u, n in enumerate([0, 0, 2, 3, 3, 3, 3, 2])}
                    om0 = 0
                    e_cur = scores_exp(*units[0])
                    for n, (b, j, qc) in enumerate(units):
                        if n + 1 < len(units):
                            bn, jn, qn = units[n + 1]
                            e_next = p5.tile([128, NKT * 512], BF16, tag="e",
                                             bufs=3, name="enx")
                            av_norm(b, j, qc, e_cur,
                                    nxt=(bn, jn, qn, e_next))
                        else:
                            e_next = None
                            av_norm(b, j, qc, e_cur)
                        e_cur = e_next
                        for _ in range(om_per_unit.get(n, 0)):
                            oproj_unit(0, om0)
                            om0 += 1
                        if qc == 3:
                            a2a_and_af(b, j)

                    # ---------- P7 tail: o_proj for batch 1.
                    # Even k16 chunks (head j=0 features) only need
                    # A2A(1,0) and run while A2A(1,1) is in flight; odd
                    # chunks + combine follow.
                    o_evs = [p7.tile([128, SPC], BF16, tag="oev", bufs=NH,
                                     name=f"oev{om}") for om in range(NH)]
                    for om in range(NH):
                        oproj_unit(1, om, phase=0, o_ev=o_evs[om])
                    for om in range(NH):
                        oproj_unit(1, om, phase=1, o_ev=o_evs[om])
            ctx_pass.close()

    nc.compile()
    return nc


def build_in_maps(hidden_states, q_w, kv_a_w, kv_b_w, o_w, kv_norm_w):
    import ml_dtypes
    BF = ml_dtypes.bfloat16

    hs = np.ascontiguousarray(np.asarray(hidden_states, dtype=np.float32))
    q_w = np.asarray(q_w, dtype=np.float32)
    kv_a_w = np.asarray(kv_a_w, dtype=np.float32)
    kv_b_w = np.asarray(kv_b_w, dtype=np.float32)
    o_w = np.asarray(o_w, dtype=np.float32)
    kv_norm_w = np.asarray(kv_norm_w, dtype=np.float32)

    hsT = np.ascontiguousarray(hs.reshape(T, HID).T).astype(BF)       # [HID, T]
    kvaT = np.ascontiguousarray(kv_a_w[ROPE:, :].T).astype(BF)        # [HID, RANK]
    scale = D ** -0.5
    cosT, sinT = _yarn_cos_sin()
    sinsh = np.concatenate([sinT[32:64], -sinT[0:32]], axis=0)
    ident = np.eye(128, dtype=np.float32).astype(BF)
    ones = np.ones((128, 128), dtype=np.float32).astype(BF)
    # owt[om, p, k*128+m] = o_w[om*128+m, k*128+p]
    owt = np.ascontiguousarray(
        o_w.reshape(16, 128, 16, 128).transpose(0, 3, 2, 1)
        .reshape(16, 128, HID)).astype(BF)

    kvb = (kv_b_w * kv_norm_w[None, :]).reshape(H, 2, D, RANK)

    in_maps = []
    for c in range(NCORES):
        qwT = np.ascontiguousarray(
            (q_w[c * HPC * D:(c + 1) * HPC * D] * scale).T).astype(BF)  # [HID, 256]
        # kvbT cols order per core: k0,k1,v0,v1 each 128 wide
        blk = kvb[c * HPC:(c + 1) * HPC]                               # [2,2,128,RANK]
        kvbT = np.ascontiguousarray(
            blk.transpose(1, 0, 2, 3).reshape(HPC * 2 * D, RANK).T).astype(BF)
        hsmy = np.ascontiguousarray(
            hsT[:, c * TPC:(c + 1) * TPC])
        in_maps.append({
            "hsT": hsT, "hsmy": hsmy, "qwT": qwT, "kvaT": kvaT,
            "kvbT": kvbT, "owt": owt,
            "cos": cosT.astype(BF), "sinsh": sinsh.astype(BF),
            "ident": ident, "ones": ones,
        })
    return in_maps


def assemble_output(results):
    out = np.empty((B, S, HID), dtype=np.float32)
    for c in range(NCORES):
        r = results[c]["outT"]                 # [HID, 2*SPC]
        out[0, c * SPC:(c + 1) * SPC, :] = r[:, 0:SPC].T
        out[1, c * SPC:(c + 1) * SPC, :] = r[:, SPC:2 * SPC].T
    return out


def kernel(hidden_states, q_w, kv_a_w, kv_b_w, o_w, kv_norm_w):
    from concourse import bass_utils

    if "nc" not in _CACHE:
        _CACHE["nc"] = build_nc()
    nc = _CACHE["nc"]
    in_maps = build_in_maps(hidden_states, q_w, kv_a_w, kv_b_w, o_w, kv_norm_w)
    res = bass_utils.run_bass_kernel_spmd(
        nc, in_maps, core_ids=list(range(NCORES)), trace=False)
    return assemble_output(res.results)
